# revision 38
# baseline (speedup 1.0000x reference)
"""Trainium2 Bass kernel for nn_BinaryMNModel (binary Markov-network clique scoring).

Math: for each batch row b,
    ll[b] = sum_c sum_j f[c,j] * prod_s ( bc[j,s] ? x[b,vars[c,s]] : 1-x[b,vars[c,s]] )

Each clique's factor table is re-expressed in the multilinear monomial basis
(an 8x8 +-1 transform of the 8 factor entries):
    score[c,b] = g0[c] + g1[c]*a0 + g2[c]*a1 + g3[c]*a2
               + g4[c]*a0*a1 + g5[c]*a0*a2 + g6[c]*a1*a2 + g7[c]*a0*a1*a2
with a_s = x[b, vars[c,s]].  Summing over cliques:
  - constant + linear terms collapse to a host-side x @ w + const (w is an
    O(V) scatter-add of the linear coefficients),
  - only the 4 quadratic/cubic monomial sums run on device.

Sharding: cliques are sharded across the 8 cores (2500 each, padded to 2560 =
20 chunks of 128).  Each core returns 12 partial [256] rows; host sums them.

Per core on device (all device data fp16; psum f32):
  - dma_gather calls pull rows of x^T [V, B] fp16 from DRAM for all 3
    slots at once (idx interleaved chunk-major/slot-minor) into
    a_all [128, 60, 256].  SWDGE descgen costs ~8.6ns/row on a queue's Q7
    pair and is the pipeline pacer; the gather ucode library load (~9us
    after the ~7us init preamble) sets the earliest descgen start (~16us).
    12 calls in 3 waves of 4 (queue order 1,2,3,0 — a queue-0 call runs
    its descgen synchronously on the Pool sequencer and would delay the
    async queues' dispatch if issued first); a call's DMA only starts
    after its descgen completes, so the last wave is small.
  - DVE computes the 4 unweighted products per chunk-block as big
    tensor_tensor ops (fp16 2x datapath, ~0.52ns/elem): p01, p02, p12,
    p012 = p01*a2.
  - PE reduces over cliques with the per-clique monomial weights as the
    stationary [128, 4] = (g01, g02, g12, g012) per chunk; moving is the
    product pair (p01|p02) resp. (p12|p012) [128, 512].  Only the
    "diagonal" entries (stationary col m) x (moving half m) are wanted;
    the off-diagonal products accumulate junk in psum rows that are never
    read.  Chunks round-robin over PE column groups 0/32/64 (col group 3
    is a known TRN2 HW bug), 2 psum banks (A: p01/p02, B: p12/p012).
  - psum rows 0:68 copy to SBUF as fp16 (DVE + ACT in parallel), one DMA
    to DRAM; host picks the 12 useful slices and sums them.
"""

import numpy as np

# ---------------------------------------------------------------- constants
B = 256
V = 5000
C = 20000
S = 3
NCOMB = 8
N_CORES = 8

C_SHARD = C // N_CORES          # 2500 cliques per core
CHUNKS = 20                     # 2560 = 20 * 128
C_PAD = CHUNKS * 128            # padded cliques per core
# gather call sizes in 128-row subcolumns, 3 waves of 4 calls over the 4
# SWDGE queues.  Wave k+1 dispatches when queue 0's wave-k call retires
# (q0 descgen runs synchronously on the Pool sequencer), so q0's chain
# sets the pacing; each queue gets 15 subcols = 1920 rows of descgen
# (~8.6ns/row).  A call's DMA only starts after its descgen completes, so
# the last wave is small (short tail).  Per-call fixed cost is ~1.2us, so
# 12 calls total; >8 calls would reuse Tile's DMASW sem lanes and wait on
# earlier DMAs.  The SWDGE ring caps a call at 1024 descriptors.
# (subcol_start, n_subcols) per call; wave 3's ranges run BACKWARD so the
# last-needed chunks ride the earliest-dispatched queues (1, 2) and the
# sequencer-synchronous queue 0 covers the wave's earliest subcols
GATHER_CALLS = [
    (0, 5), (5, 5), (10, 5), (15, 5),
    (20, 6), (26, 6), (32, 6), (38, 6),
    (56, 4), (52, 4), (48, 4), (44, 4),
]
assert sorted(
    s for s0, n in GATHER_CALLS for s in range(s0, s0 + n)
) == list(range(3 * CHUNKS)) and max(n for _, n in GATHER_CALLS) * 128 <= 1024
GATHER_QUEUE = [1, 2, 3, 0] * 3
# product blocks (chunks): first needs only wave 1; small last blocks
GROUP_CHUNKS = [5, 5, 4, 2, 2, 2]
assert sum(GROUP_CHUNKS) == CHUNKS
N_QUEUES = 4
# interleaved idx: position j = (chunk*3 + slot)*128 + p -> 16-wrap cols
IDX_COLS = 3 * C_PAD // 16      # 480 columns total

_PROGRAM = None


def _build_program():
    import concourse.mybir as mybir
    from concourse import bacc, tile

    f32 = mybir.dt.float32
    f16 = mybir.dt.float16
    i16 = mybir.dt.int16
    MULT = mybir.AluOpType.mult

    nc = bacc.Bacc(
        "TRN2",
        target_bir_lowering=False,
        debug=False,
        enable_asserts=False,
        num_devices=N_CORES,
        num_swdge_queues=N_QUEUES,
    )

    xt_d = nc.dram_tensor("xt", [V, B], f16, kind="ExternalInput")
    idx_d = nc.dram_tensor("idx", [128, IDX_COLS], i16, kind="ExternalInput")
    gq_d = nc.dram_tensor("gq", [128, CHUNKS * 4], f16, kind="ExternalInput")
    out_d = nc.dram_tensor("out", [68, 2, 512], f16, kind="ExternalOutput")

    with tile.TileContext(nc) as tc:
        with (
            tc.tile_pool(name="persist", bufs=1) as pp,
            tc.tile_pool(name="prod", bufs=3) as prodp,
            tc.tile_pool(name="ps", bufs=1, space="PSUM") as psp,
        ):
            idx_t = pp.tile([128, IDX_COLS], i16, tag="idx")
            gq_t = pp.tile([128, CHUNKS, 4], f16, tag="gq")
            # a_all[:, 3*c + s, :] = x[vars[128c+p, s], :] for partition p
            a_all = pp.tile([128, 3 * CHUNKS, B], f16, tag="a_all")
            psA = psp.tile([128, 512], f32, tag="psA")
            psB = psp.tile([128, 512], f32, tag="psB")

            # idx loads split so the first gather call only waits on a small
            # fast DMA (the completion-sem propagation is several us; a
            # monolithic idx load gates the whole descgen pipeline)
            nc.sync.dma_start(idx_t[:, 0:64], idx_d[:, 0:64])
            nc.sync.dma_start(idx_t[:, 64:IDX_COLS], idx_d[:, 64:IDX_COLS])
            nc.sync.dma_start(gq_t[:], gq_d[:])

            # zero the psum banks so the final full-width copies never read
            # uninitialized partitions (only 12 rows are matmul targets)
            nc.vector.memset(psA[:], 0.0)
            nc.vector.memset(psB[:], 0.0)


            # all gathers up front; they pace everything downstream
            for i, (sc0, nsc) in enumerate(GATHER_CALLS):
                sc1 = sc0 + nsc
                n_idx = nsc * 128
                nc.gpsimd.dma_gather(
                    a_all[:, sc0:sc1, :],
                    xt_d[:],
                    idx_t[:, sc0 * 8 : sc1 * 8],
                    n_idx,
                    n_idx,
                    B,
                    queue_num=GATHER_QUEUE[i],
                )
            c_off = [0]
            for gc in GROUP_CHUNKS:
                c_off.append(c_off[-1] + gc)

            # PE chain bookkeeping: 6 chains = (bank A/B) x (col group
            # 0/32/64); chunk c -> col group 32*(c%3).
            started = set()
            grp_chunks = {r: [c for c in range(CHUNKS) if c % 3 == r] for r in range(3)}
            last_of = {}
            for r, cs in grp_chunks.items():
                last_of[r] = cs[-1]

            for g, gc in enumerate(GROUP_CHUNKS):
                c0, c1 = c_off[g], c_off[g + 1]
                a0 = a_all[:, 3 * c0 + 0 : 3 * c1 : 3, :]
                a1 = a_all[:, 3 * c0 + 1 : 3 * c1 : 3, :]
                a2 = a_all[:, 3 * c0 + 2 : 3 * c1 : 3, :]
                p_blk = prodp.tile([128, gc, 4, B], f16, tag="p", name=f"p{g}")
                nc.vector.tensor_tensor(p_blk[:, :, 0, :], a0, a1, MULT)
                nc.vector.tensor_tensor(p_blk[:, :, 1, :], a0, a2, MULT)
                nc.vector.tensor_tensor(p_blk[:, :, 2, :], a1, a2, MULT)
                nc.vector.tensor_tensor(
                    p_blk[:, :, 3, :], p_blk[:, :, 0, :], a2, MULT
                )
                for ci in range(gc):
                    c = c0 + ci
                    r = c % 3
                    row = 32 * r
                    lhs = gq_t[:, c, :]
                    for bank, ps_t in (("A", psA), ("B", psB)):
                        half = 0 if bank == "A" else 2
                        key = (bank, r)
                        nc.tensor.matmul(
                            ps_t[row : row + 4, :],
                            lhs,
                            p_blk[:, ci, half : half + 2, :],
                            start=(key not in started),
                            stop=(c == last_of[r]),
                            tile_position=(0, row),
                        )
                        started.add(key)

            # psum rows 0:68 -> one sbuf tile as fp16 (partition-preserving;
            # DMA can't read PSUM) on DVE + ACT in parallel, then a single
            # DMA -> DRAM; host picks out the 12 useful slices and sums them.
            sbAB = pp.tile([68, 2, 512], f16, tag="sbAB")
            nc.vector.tensor_copy(sbAB[:, 0, :], psA[0:68, :])
            nc.scalar.activation(
                sbAB[:, 1, :], psB[0:68, :], mybir.ActivationFunctionType.Copy
            )
            nc.sync.dma_start(out_d[:], sbAB[:])

    nc.compile()
    return nc, out_d.name


def get_program():
    global _PROGRAM
    if _PROGRAM is None:
        _PROGRAM = _build_program()
    return _PROGRAM


# ---------------------------------------------------------------- host prep
def _monomial_transform(all_factors: np.ndarray) -> np.ndarray:
    """g[c,t] such that score[c,b] = sum_t g[c,t] * prod_{s: bit (S-1-s) of t} a_s."""
    M = np.zeros((NCOMB, NCOMB), dtype=np.float64)
    for t in range(NCOMB):
        for j in range(NCOMB):
            if j & ~t:
                continue
            M[t, j] = (-1.0) ** bin(t & ~j).count("1")
    return all_factors.astype(np.float64) @ M.T


def prepare_inputs(x, all_vars, all_factors):
    x = np.asarray(x, dtype=np.float32)
    all_vars = np.asarray(all_vars)
    all_factors = np.asarray(all_factors, dtype=np.float32)

    xt16 = np.ascontiguousarray(x.T.astype(np.float16))  # [V, B]

    g = _monomial_transform(all_factors)  # [C, 8] f64
    bit = [1 << (S - 1 - s) for s in range(S)]
    t01, t02, t12 = bit[0] | bit[1], bit[0] | bit[2], bit[1] | bit[2]
    t012 = bit[0] | bit[1] | bit[2]

    # constant + linear terms on host
    w = np.zeros(V, dtype=np.float64)
    for s in range(S):
        np.add.at(w, all_vars[:, s], g[:, bit[s]])
    base = g[:, 0].sum() + x.astype(np.float64) @ w  # [B]

    in_maps = []
    for k in range(N_CORES):
        sl = slice(k * C_SHARD, (k + 1) * C_SHARD)
        pad = C_PAD - C_SHARD

        # idx interleaved: position j = (c*3 + s)*128 + p -> clique 128c+p slot s
        av = np.concatenate(
            [all_vars[sl], np.zeros((pad, S), np.int64)], axis=0
        )  # [C_PAD, 3]
        # [C_PAD, 3] -> positions (c, s, p): reshape cliques (c,p)
        avr = av.reshape(CHUNKS, 128, S).transpose(0, 2, 1)  # [chunk, slot, p]
        flat = avr.reshape(-1)  # position j
        wrapped = flat.reshape(IDX_COLS, 16).T.astype(np.int16)  # [16, IDX_COLS]
        idx_arr = np.ascontiguousarray(np.tile(wrapped, (8, 1)))

        # gq [128, CHUNKS*4] fp16: partition p, col c*4+m = g_mono[m][128c+p]
        gq = np.zeros((128, CHUNKS, 4), dtype=np.float16)
        for m, t in enumerate((t01, t02, t12, t012)):
            gg = np.concatenate([g[sl, t], np.zeros(pad)]).reshape(CHUNKS, 128)
            gq[:, :, m] = gg.T.astype(np.float16)
        gq_arr = np.ascontiguousarray(gq.reshape(128, CHUNKS * 4))

        in_maps.append({"xt": xt16, "idx": idx_arr, "gq": gq_arr})

    return in_maps, base.astype(np.float64)


# ---------------------------------------------------------------- entry
def run(inputs: dict, trace: bool = False):
    from concourse import bass_utils

    in_maps, base = prepare_inputs(
        inputs["x"], inputs["all_vars"], inputs["all_factors"]
    )
    nc, out_name = get_program()
    res = bass_utils.run_bass_kernel_spmd(
        nc, in_maps, core_ids=list(range(N_CORES)), trace=trace
    )
    acc = base.copy()
    for r in res.results:
        o = np.asarray(r[out_name]).astype(np.float64)  # [68, 2, 512]
        for g in (0, 32, 64):
            acc += o[g + 0, 0, 0:B] + o[g + 1, 0, B:]
            acc += o[g + 2, 1, 0:B] + o[g + 3, 1, B:]
    return acc.astype(np.float32), res


def kernel(x, binary_combinations, all_vars, all_factors):
    out, _ = run({"x": x, "all_vars": all_vars, "all_factors": all_factors})
    return out



# revision 41
# speedup vs baseline: 1.0483x; 1.0483x over previous
"""Trainium2 Bass kernel for nn_BinaryMNModel (binary Markov-network clique scoring).

Math: for each batch row b,
    ll[b] = sum_c sum_j f[c,j] * prod_s ( bc[j,s] ? x[b,vars[c,s]] : 1-x[b,vars[c,s]] )

Each clique's factor table is re-expressed in the multilinear monomial basis
(an 8x8 +-1 transform of the 8 factor entries):
    score[c,b] = g0[c] + g1[c]*a0 + g2[c]*a1 + g3[c]*a2
               + g4[c]*a0*a1 + g5[c]*a0*a2 + g6[c]*a1*a2 + g7[c]*a0*a1*a2
with a_s = x[b, vars[c,s]].  Summing over cliques:
  - constant + linear terms collapse to a host-side x @ w + const (w is an
    O(V) scatter-add of the linear coefficients),
  - only the 4 quadratic/cubic monomial sums run on device.

Sharding: cliques are sharded across the 8 cores (2500 each, padded to 2560 =
20 chunks of 128).  Each core returns 12 partial [256] rows; host sums them.

Per core on device (all device data fp16; psum f32):
  - dma_gather calls pull rows of x^T [V, B] fp16 from DRAM for all 3
    slots at once (idx interleaved chunk-major/slot-minor) into
    a_all [128, 60, 256].  SWDGE descgen costs ~8.6ns/row on a queue's Q7
    pair and is the pipeline pacer; the gather ucode library load (~9us
    after the ~7us init preamble) sets the earliest descgen start (~16us).
    12 calls in 3 waves of 4 (queue order 1,2,3,0 — a queue-0 call runs
    its descgen synchronously on the Pool sequencer and would delay the
    async queues' dispatch if issued first); a call's DMA only starts
    after its descgen completes, so the last wave is small.
  - DVE computes the 4 unweighted products per chunk-block as big
    tensor_tensor ops (fp16 2x datapath, ~0.52ns/elem): p01, p02, p12,
    p012 = p01*a2.
  - PE reduces over cliques with the per-clique monomial weights as the
    stationary [128, 4] = (g01, g02, g12, g012) per chunk; moving is the
    product pair (p01|p02) resp. (p12|p012) [128, 512].  Only the
    "diagonal" entries (stationary col m) x (moving half m) are wanted;
    the off-diagonal products accumulate junk in psum rows that are never
    read.  Chunks round-robin over PE column groups 0/32/64 (col group 3
    is a known TRN2 HW bug), 2 psum banks (A: p01/p02, B: p12/p012).
  - psum rows 0:68 copy to SBUF as fp16 (DVE + ACT in parallel), one DMA
    to DRAM; host picks the 12 useful slices and sums them.
"""

import numpy as np

# ---------------------------------------------------------------- constants
B = 256
V = 5000
C = 20000
S = 3
NCOMB = 8
N_CORES = 8

C_SHARD = C // N_CORES          # 2500 cliques per core
CHUNKS = 20                     # 2560 = 20 * 128
C_PAD = CHUNKS * 128            # padded cliques per core
# gather call sizes in 128-row subcolumns, 3 waves of 4 calls over the 4
# SWDGE queues.  Wave k+1 dispatches when queue 0's wave-k call retires
# (q0 descgen runs synchronously on the Pool sequencer), so q0's chain
# sets the pacing; each queue gets 15 subcols = 1920 rows of descgen
# (~8.6ns/row).  A call's DMA only starts after its descgen completes, so
# the last wave is small (short tail).  Per-call fixed cost is ~1.2us, so
# 12 calls total; >8 calls would reuse Tile's DMASW sem lanes and wait on
# earlier DMAs.  The SWDGE ring caps a call at 1024 descriptors.
# (subcol_start, n_subcols) per call; wave 3's ranges run BACKWARD so the
# last-needed chunks ride the earliest-dispatched queues (1, 2) and the
# sequencer-synchronous queue 0 covers the wave's earliest subcols
GATHER_CALLS = [
    (0, 3), (3, 3), (6, 3), (9, 3),
    (12, 6), (18, 6), (24, 6), (30, 6),
    (54, 6), (48, 6), (42, 6), (36, 6),
]
assert sorted(
    s for s0, n in GATHER_CALLS for s in range(s0, s0 + n)
) == list(range(3 * CHUNKS)) and max(n for _, n in GATHER_CALLS) * 128 <= 1024
GATHER_QUEUE = [1, 2, 3, 0] * 3
# product blocks (chunks): first needs only wave 1; small last blocks
GROUP_CHUNKS = [4, 4, 4, 4, 2, 2]
assert sum(GROUP_CHUNKS) == CHUNKS
N_QUEUES = 4
# interleaved idx: position j = (chunk*3 + slot)*128 + p -> 16-wrap cols
IDX_COLS = 3 * C_PAD // 16      # 480 columns total

_PROGRAM = None


def _build_program():
    import concourse.mybir as mybir
    from concourse import bacc, tile

    f32 = mybir.dt.float32
    f16 = mybir.dt.float16
    i16 = mybir.dt.int16
    MULT = mybir.AluOpType.mult

    nc = bacc.Bacc(
        "TRN2",
        target_bir_lowering=False,
        debug=False,
        enable_asserts=False,
        num_devices=N_CORES,
        num_swdge_queues=N_QUEUES,
    )

    xt_d = nc.dram_tensor("xt", [V, B], f16, kind="ExternalInput")
    idx_d = nc.dram_tensor("idx", [128, IDX_COLS], i16, kind="ExternalInput")
    gq_d = nc.dram_tensor("gq", [128, CHUNKS * 4], f16, kind="ExternalInput")
    out_d = nc.dram_tensor("out", [68, 2, 512], f16, kind="ExternalOutput")

    with tile.TileContext(nc) as tc:
        with (
            tc.tile_pool(name="persist", bufs=1) as pp,
            tc.tile_pool(name="prod", bufs=2) as prodp,
            tc.tile_pool(name="ps", bufs=1, space="PSUM") as psp,
        ):
            idx_t = pp.tile([128, IDX_COLS], i16, tag="idx")
            gq_t = pp.tile([128, CHUNKS, 4], f16, tag="gq")
            # a_all[:, 3*c + s, :] = x[vars[128c+p, s], :] for partition p
            a_all = pp.tile([128, 3 * CHUNKS, B], f16, tag="a_all")
            psA = psp.tile([128, 512], f32, tag="psA")
            psB = psp.tile([128, 512], f32, tag="psB")

            # idx loads split so the first gather call only waits on a small
            # fast DMA (the completion-sem propagation is several us; a
            # monolithic idx load gates the whole descgen pipeline)
            nc.sync.dma_start(idx_t[:, 0:64], idx_d[:, 0:64])
            nc.sync.dma_start(idx_t[:, 64:IDX_COLS], idx_d[:, 64:IDX_COLS])
            nc.sync.dma_start(gq_t[:], gq_d[:])

            # zero the psum banks so the final full-width copies never read
            # uninitialized partitions (only 12 rows are matmul targets)
            nc.vector.memset(psA[:], 0.0)
            nc.vector.memset(psB[:], 0.0)


            # all gathers up front; they pace everything downstream
            for i, (sc0, nsc) in enumerate(GATHER_CALLS):
                sc1 = sc0 + nsc
                n_idx = nsc * 128
                nc.gpsimd.dma_gather(
                    a_all[:, sc0:sc1, :],
                    xt_d[:],
                    idx_t[:, sc0 * 8 : sc1 * 8],
                    n_idx,
                    n_idx,
                    B,
                    queue_num=GATHER_QUEUE[i],
                )
            c_off = [0]
            for gc in GROUP_CHUNKS:
                c_off.append(c_off[-1] + gc)

            # PE chain bookkeeping: 6 chains = (bank A/B) x (col group
            # 0/32/64); chunk c -> col group 32*(c%3).
            started = set()
            grp_chunks = {r: [c for c in range(CHUNKS) if c % 3 == r] for r in range(3)}
            last_of = {}
            for r, cs in grp_chunks.items():
                last_of[r] = cs[-1]

            for g, gc in enumerate(GROUP_CHUNKS):
                c0, c1 = c_off[g], c_off[g + 1]
                a0 = a_all[:, 3 * c0 + 0 : 3 * c1 : 3, :]
                a1 = a_all[:, 3 * c0 + 1 : 3 * c1 : 3, :]
                a2 = a_all[:, 3 * c0 + 2 : 3 * c1 : 3, :]
                p_blk = prodp.tile([128, gc, 4, B], f16, tag="p", name=f"p{g}")
                nc.vector.tensor_tensor(p_blk[:, :, 0, :], a0, a1, MULT)
                nc.vector.tensor_tensor(p_blk[:, :, 1, :], a0, a2, MULT)
                nc.vector.tensor_tensor(p_blk[:, :, 2, :], a1, a2, MULT)
                nc.vector.tensor_tensor(
                    p_blk[:, :, 3, :], p_blk[:, :, 0, :], a2, MULT
                )
                for ci in range(gc):
                    c = c0 + ci
                    r = c % 3
                    row = 32 * r
                    lhs = gq_t[:, c, :]
                    for bank, ps_t in (("A", psA), ("B", psB)):
                        half = 0 if bank == "A" else 2
                        key = (bank, r)
                        nc.tensor.matmul(
                            ps_t[row : row + 4, :],
                            lhs,
                            p_blk[:, ci, half : half + 2, :],
                            start=(key not in started),
                            stop=(c == last_of[r]),
                            tile_position=(0, row),
                        )
                        started.add(key)

            # psum rows 0:68 -> one sbuf tile as fp16 (partition-preserving;
            # DMA can't read PSUM) on DVE + ACT in parallel, then a single
            # DMA -> DRAM; host picks out the 12 useful slices and sums them.
            sbAB = pp.tile([68, 2, 512], f16, tag="sbAB")
            nc.vector.tensor_copy(sbAB[:, 0, :], psA[0:68, :])
            nc.scalar.activation(
                sbAB[:, 1, :], psB[0:68, :], mybir.ActivationFunctionType.Copy
            )
            nc.sync.dma_start(out_d[:], sbAB[:])

    nc.compile()
    return nc, out_d.name


def get_program():
    global _PROGRAM
    if _PROGRAM is None:
        _PROGRAM = _build_program()
    return _PROGRAM


# ---------------------------------------------------------------- host prep
def _monomial_transform(all_factors: np.ndarray) -> np.ndarray:
    """g[c,t] such that score[c,b] = sum_t g[c,t] * prod_{s: bit (S-1-s) of t} a_s."""
    M = np.zeros((NCOMB, NCOMB), dtype=np.float64)
    for t in range(NCOMB):
        for j in range(NCOMB):
            if j & ~t:
                continue
            M[t, j] = (-1.0) ** bin(t & ~j).count("1")
    return all_factors.astype(np.float64) @ M.T


def prepare_inputs(x, all_vars, all_factors):
    x = np.asarray(x, dtype=np.float32)
    all_vars = np.asarray(all_vars)
    all_factors = np.asarray(all_factors, dtype=np.float32)

    xt16 = np.ascontiguousarray(x.T.astype(np.float16))  # [V, B]

    g = _monomial_transform(all_factors)  # [C, 8] f64
    bit = [1 << (S - 1 - s) for s in range(S)]
    t01, t02, t12 = bit[0] | bit[1], bit[0] | bit[2], bit[1] | bit[2]
    t012 = bit[0] | bit[1] | bit[2]

    # constant + linear terms on host
    w = np.zeros(V, dtype=np.float64)
    for s in range(S):
        np.add.at(w, all_vars[:, s], g[:, bit[s]])
    base = g[:, 0].sum() + x.astype(np.float64) @ w  # [B]

    in_maps = []
    for k in range(N_CORES):
        sl = slice(k * C_SHARD, (k + 1) * C_SHARD)
        pad = C_PAD - C_SHARD

        # idx interleaved: position j = (c*3 + s)*128 + p -> clique 128c+p slot s
        av = np.concatenate(
            [all_vars[sl], np.zeros((pad, S), np.int64)], axis=0
        )  # [C_PAD, 3]
        # [C_PAD, 3] -> positions (c, s, p): reshape cliques (c,p)
        avr = av.reshape(CHUNKS, 128, S).transpose(0, 2, 1)  # [chunk, slot, p]
        flat = avr.reshape(-1)  # position j
        wrapped = flat.reshape(IDX_COLS, 16).T.astype(np.int16)  # [16, IDX_COLS]
        idx_arr = np.ascontiguousarray(np.tile(wrapped, (8, 1)))

        # gq [128, CHUNKS*4] fp16: partition p, col c*4+m = g_mono[m][128c+p]
        gq = np.zeros((128, CHUNKS, 4), dtype=np.float16)
        for m, t in enumerate((t01, t02, t12, t012)):
            gg = np.concatenate([g[sl, t], np.zeros(pad)]).reshape(CHUNKS, 128)
            gq[:, :, m] = gg.T.astype(np.float16)
        gq_arr = np.ascontiguousarray(gq.reshape(128, CHUNKS * 4))

        in_maps.append({"xt": xt16, "idx": idx_arr, "gq": gq_arr})

    return in_maps, base.astype(np.float64)


# ---------------------------------------------------------------- entry
def run(inputs: dict, trace: bool = False):
    from concourse import bass_utils

    in_maps, base = prepare_inputs(
        inputs["x"], inputs["all_vars"], inputs["all_factors"]
    )
    nc, out_name = get_program()
    res = bass_utils.run_bass_kernel_spmd(
        nc, in_maps, core_ids=list(range(N_CORES)), trace=trace
    )
    acc = base.copy()
    for r in res.results:
        o = np.asarray(r[out_name]).astype(np.float64)  # [68, 2, 512]
        for g in (0, 32, 64):
            acc += o[g + 0, 0, 0:B] + o[g + 1, 0, B:]
            acc += o[g + 2, 1, 0:B] + o[g + 3, 1, B:]
    return acc.astype(np.float32), res


def kernel(x, binary_combinations, all_vars, all_factors):
    out, _ = run({"x": x, "all_vars": all_vars, "all_factors": all_factors})
    return out

